# revision 21
# baseline (speedup 1.0000x reference)
"""Trainium2 Bass kernel for nn_ActionScoringModel (LRU + max-pool + tanh MLP).

Strategy: data-parallel over batch (64 = 8 cores x 8 batches). No collectives.
Per core (V2.1 pipeline):
  - obs/act cast to bf16 AND transposed on host -> obsT [NB, 3, 128, S],
    actT [3, 128, NB*A]; device does plain contiguous DMA loads only.
  - u = statA @ obsT, k-major stationary reuse (PSUM, 4 banks)
  - rotate-in: uAc = bf16 copy of u (Act); t1 = uAc (.) cos (DVE),
    t2 = uAc (.) sin' (gpsimd); v = I@t1 + P@t2 on PE (partition swap folded
    into permutation stationary P), Act copies v -> SBUF (padded by 1 col).
  - scan decimation x2: wE_m = rho v_{2m-1} + v_{2m} (stt on DVE);
    hardware scan of length 1024 with rho^2 (DVE); odd positions fixed up
    with one stt: gO = rho gE + v_odd. g layout = [even plane | odd plane]
    which is fine because latent = max over s (permutation invariant).
  - rotate-out: p1 = g (.) cos2P, p2 = g (.) sin2P (plane-ordered tables)
  - y = CM1@p1 + CM2@p2 + D@obsT(strided) per 512-block; two blocks share
    one PSUM bank (partitions 0:64 / 64:128) so each max-reduce covers two
    blocks; final cross-partition pair-max via P matmul + tensor MAX.
  - tanh MLP head on [latent, act].
"""

import sys
import numpy as np
from contextlib import ExitStack

for _p in ("/opt/trn_rl_repo",):
    if _p not in sys.path:
        sys.path.insert(0, _p)

import ml_dtypes
import concourse.bass as bass
import concourse.tile as tile
from concourse import bacc, mybir
from concourse.bass_utils import run_bass_kernel_spmd

BF16 = mybir.dt.bfloat16
F32 = mybir.dt.float32

B_, S_, A_, D_IN, H_, D_OUT, D_MLP = 64, 2048, 128, 384, 64, 64, 64
NCORES = 8
NB = B_ // NCORES          # 8 batches per core
NSB = S_ // 512            # 4 s-blocks of 512
NDC = D_IN // 128          # 3 d-chunks
SH = S_ // 2               # 1024, scan length / plane width


def _build_nc():
    nc = bacc.Bacc("TRN2", target_bir_lowering=False, debug=False,
                   num_devices=1)

    # ---- DRAM I/O ----
    obsT_d = nc.dram_tensor("obsT", [NB, NDC, 128, S_], BF16,
                            kind="ExternalInput").ap()
    actT_d = nc.dram_tensor("actT", [NDC, 128, NB * A_], BF16,
                            kind="ExternalInput").ap()
    tabs_d = nc.dram_tensor("tabs", [128, 3 * S_], BF16, kind="ExternalInput").ap()
    rhopk_d = nc.dram_tensor("rhopk", [128, SH + 1], F32, kind="ExternalInput").ap()
    statpk_d = nc.dram_tensor("statpk", [128, 1600], BF16, kind="ExternalInput").ap()
    w2_d = nc.dram_tensor("w2", [64, 32], BF16, kind="ExternalInput").ap()
    w3_d = nc.dram_tensor("w3", [32, 1], BF16, kind="ExternalInput").ap()
    b1_d = nc.dram_tensor("b1", [64, 1], F32, kind="ExternalInput").ap()
    b2_d = nc.dram_tensor("b2", [32, 1], F32, kind="ExternalInput").ap()
    b3_d = nc.dram_tensor("b3", [1, 1], F32, kind="ExternalInput").ap()
    out_d = nc.dram_tensor("out", [1, NB * A_], F32, kind="ExternalOutput").ap()

    MULT = mybir.AluOpType.mult
    ADD = mybir.AluOpType.add
    MAX = mybir.AluOpType.max
    TANH = mybir.ActivationFunctionType.Tanh
    X = mybir.AxisListType.X

    with tile.TileContext(nc) as tc, ExitStack() as ctx:
        const = ctx.enter_context(tc.tile_pool(name="const", bufs=1))
        obsT_pool = ctx.enter_context(tc.tile_pool(name="obsT", bufs=3))
        work = ctx.enter_context(tc.tile_pool(name="work", bufs=2))
        tpool = ctx.enter_context(tc.tile_pool(name="tpool", bufs=3))
        pUA = ctx.enter_context(tc.tile_pool(name="pUA", bufs=1, space="PSUM"))
        pWE = ctx.enter_context(tc.tile_pool(name="pWE", bufs=1, space="PSUM"))
        pY = ctx.enter_context(tc.tile_pool(name="pY", bufs=1, space="PSUM"))
        small = ctx.enter_context(tc.tile_pool(name="small", bufs=1))

        def load_const(ap_d, shape, dtype, suffix=""):
            nm = f"c_{ap_d.tensor.name}{suffix}"
            t = const.tile(shape, dtype, tag=nm, name=nm)
            nc.scalar.dma_start(out=t[:], in_=ap_d)
            return t

        # packed consts: stationaries first (small, unblock compute), then
        # big tables split across both hwdge queues
        statpk = const.tile([128, 1600], BF16, tag="statpk", name="statpk")
        nc.scalar.dma_start(out=statpk[:], in_=statpk_d)
        rhopk = const.tile([128, SH + 1], F32, tag="rhopk", name="rhopk")
        nc.scalar.dma_start(out=rhopk[:], in_=rhopk_d)
        tabs = const.tile([128, 3 * S_], BF16, tag="tabs", name="tabs")
        w2 = load_const(w2_d, [64, 32], BF16)
        w3 = load_const(w3_d, [32, 1], BF16)
        b1 = load_const(b1_d, [64, 1], F32)
        b2 = load_const(b2_d, [32, 1], F32)
        b3 = load_const(b3_d, [1, 1], F32)

        cosS = tabs[:, 0:S_]
        sinpm2 = tabs[:, S_:2 * S_]
        cosE = tabs[:, 2 * S_:2 * S_ + SH]
        sinE = tabs[:, 2 * S_ + SH:3 * S_]
        rho2f = rhopk[:, 0:SH]
        rho1 = rhopk[:, SH:SH + 1]
        statA = [statpk[:, k * 128:(k + 1) * 128] for k in range(NDC)]
        permP = statpk[:, 384:512]
        ident = statpk[:, 512:640]
        statD = [statpk[:, 640 + k * 64:640 + (k + 1) * 64] for k in range(NDC)]
        cm1 = statpk[:, 832:896]
        cm2 = statpk[:, 896:960]
        w1lat = statpk[:, 960:1024]
        w1act = [statpk[:, 1024 + k * 64:1024 + (k + 1) * 64] for k in range(NDC)]
        rhoI = statpk[:, 1216:1344]
        rhoP = statpk[:, 1344:1472]
        cm1l = statpk[:, 1472:1536]
        cm2l = statpk[:, 1536:1600]

        lat128 = small.tile([128, NB], F32)     # per-pair latent maxima

        # prefetch batch-0 obsT ahead of everything else on the sync queue
        obsT0 = [obsT_pool.tile([128, S_], BF16, tag=f"obsT{k}",
                                name=f"obsT{k}") for k in range(NDC)]
        for k in range(NDC):
            nc.sync.dma_start(out=obsT0[k][:], in_=obsT_d[0, k])

        # action-side MLP input (independent of the LRU path): compute
        # xa = W1act @ actT early so the tail only needs activations
        actT = [small.tile([128, NB * A_], BF16, tag=f"actT{k}",
                           name=f"actT{k}") for k in range(NDC)]
        for k in range(NDC):
            nc.scalar.dma_start(out=actT[k][:], in_=actT_d[k])
        nc.scalar.dma_start(out=tabs[:, 0:2 * S_], in_=tabs_d[:, 0:2 * S_])
        nc.sync.dma_start(out=tabs[:, 2 * S_:], in_=tabs_d[:, 2 * S_:])
        xa = small.tile([64, NB * A_], F32, tag="xa", name="xa")
        lat128b = small.tile([128, NB], BF16)
        latf = small.tile([64, NB], F32)
        latb = small.tile([64, NB], BF16)
        latWb = small.tile([64, NB], F32)
        x1 = small.tile([64, NB * A_], BF16)
        x2 = small.tile([32, NB * A_], BF16)
        x3 = small.tile([1, NB * A_], F32)

        def emit_mlp_half(h):
            bs = slice(h * (NB // 2), (h + 1) * (NB // 2))
            hl = slice(h * 512, (h + 1) * 512)
            nc.vector.tensor_copy(out=lat128b[:, bs], in_=lat128[:, bs])
            pswap = pWE.tile([128, 512], F32, tag="wE0", name="pswap")
            nc.tensor.matmul(out=pswap[:, 0:NB // 2], lhsT=permP,
                             rhs=lat128b[:, bs], start=True, stop=True)
            nc.vector.tensor_tensor(out=latf[:, bs], in0=lat128[0:64, bs],
                                    in1=pswap[0:64, 0:NB // 2], op=MAX)
            nc.vector.tensor_copy(out=latb[:, bs], in_=latf[:, bs])
            platW = pWE.tile([128, 512], F32, tag="wE1", name="platW")
            nc.tensor.matmul(out=platW[:64, 0:NB // 2], lhsT=w1lat[0:64, :],
                             rhs=latb[:, bs], start=True, stop=True)
            nc.vector.tensor_scalar(out=latWb[:, bs],
                                    in0=platW[:64, 0:NB // 2],
                                    scalar1=b1[:], scalar2=None, op0=ADD)
            for bb in range(NB // 2):
                b_idx = h * 4 + bb
                nc.scalar.activation(
                    out=x1[:, b_idx * A_:(b_idx + 1) * A_],
                    in_=xa[:, b_idx * A_:(b_idx + 1) * A_],
                    func=TANH, bias=latWb[:, b_idx:b_idx + 1], scale=1.0)
            px = pWE.tile([128, 512], F32, tag="wE0", name="px2")
            nc.tensor.matmul(out=px[:32, :], lhsT=w2[:], rhs=x1[:, hl],
                             start=True, stop=True)
            nc.scalar.activation(out=x2[:, hl], in_=px[:32, :], func=TANH,
                                 bias=b2[:], scale=1.0)
            px3 = pWE.tile([128, 512], F32, tag="wE1", name="px3")
            nc.tensor.matmul(out=px3[:1, :], lhsT=w3[:], rhs=x2[:, hl],
                             start=True, stop=True)
            nc.scalar.activation(out=x3[:, hl], in_=px3[:1, :], func=TANH,
                                 bias=b3[:], scale=1.0)

        def emit_xa():
            for half in range(2):
                hl = slice(half * 512, (half + 1) * 512)
                pxa = pWE.tile([128, 512], F32, tag="wE0", name="pxa")
                for k in range(NDC):
                    nc.tensor.matmul(out=pxa[:64, :], lhsT=w1act[k],
                                     rhs=actT[k][:, hl],
                                     start=(k == 0), stop=(k == NDC - 1))
                nc.scalar.copy(out=xa[:, hl], in_=pxa[:64, :])

        # ---------------- main loop over local batches ----------------
        for b in range(NB):
            if b == 0:
                obsT = obsT0
            else:
                obsT = [obsT_pool.tile([128, S_], BF16, tag=f"obsT{k}",
                                       name=f"obsT{k}")
                        for k in range(NDC)]
                for k in range(NDC):
                    nc.sync.dma_start(out=obsT[k][:], in_=obsT_d[b, k])

            # u = statA @ obsT, k-major (3 weight loads per batch)
            uA = [None] * NSB
            for k in range(NDC):
                for i in range(NSB):
                    if k == 0:
                        uA[i] = pUA.tile([128, 512], F32, tag=f"uA{i}",
                                         name=f"uA{i}")
                    nc.tensor.matmul(
                        out=uA[i][:], lhsT=statA[k],
                        rhs=obsT[k][:, i * 512:(i + 1) * 512],
                        start=(k == 0), stop=(k == NDC - 1))

            # rotate-in into padded full-batch tiles (col0 = 0)
            t1 = work.tile([128, S_ + 1], BF16, tag="t1", name="t1")
            t2 = work.tile([128, S_ + 1], BF16, tag="t2", name="t2")
            nc.gpsimd.memset(t1[:, 0:1], 0.0)
            nc.gpsimd.memset(t2[:, 0:1], 0.0)
            uAc = work.tile([128, S_], BF16, tag="uAc", name="uAc")
            for i in range(NSB):
                sl = slice(i * 512, (i + 1) * 512)
                slp = slice(1 + i * 512, 1 + (i + 1) * 512)
                nc.scalar.copy(out=uAc[:, sl], in_=uA[i][:])
                nc.vector.tensor_tensor(out=t1[:, slp], in0=uA[i][:],
                                        in1=cosS[:, sl], op=MULT)
                nc.gpsimd.tensor_tensor(out=t2[:, slp], in0=uAc[:, sl],
                                        in1=sinpm2[:, sl], op=MULT)

            # wE = rhoI@t1_odd + rhoP@t2_odd + I@t1_even + P@t2_even on PE
            # (wE_m = rho*v_{2m-1} + v_{2m}, v = I@t1 + P@t2)
            t1_lo = t1[:, 0:S_].rearrange("p (n f) -> p f n", f=2)[:, 0]
            t1_hi = t1[:, 1:S_ + 1].rearrange("p (n f) -> p f n", f=2)[:, 0]
            t2_lo = t2[:, 0:S_].rearrange("p (n f) -> p f n", f=2)[:, 0]
            t2_hi = t2[:, 1:S_ + 1].rearrange("p (n f) -> p f n", f=2)[:, 0]
            wE = [pWE.tile([128, 512], F32, tag=f"wE{j}", name=f"wE{j}")
                  for j in range(2)]
            for j in range(2):
                jl = slice(j * 512, (j + 1) * 512)
                nc.tensor.matmul(out=wE[j][:], lhsT=rhoI, rhs=t1_lo[:, jl],
                                 start=True, stop=False)
                nc.tensor.matmul(out=wE[j][:], lhsT=rhoP, rhs=t2_lo[:, jl],
                                 start=False, stop=False)
                nc.tensor.matmul(out=wE[j][:], lhsT=ident, rhs=t1_hi[:, jl],
                                 start=False, stop=False)
                nc.tensor.matmul(out=wE[j][:], lhsT=permP, rhs=t2_hi[:, jl],
                                 start=False, stop=True)

            # chained length-512 scans with rho^2 over the wE PSUM banks
            g = work.tile([128, SH], BF16, tag="g", name="g")
            nc.vector.tensor_tensor_scan(out=g[:, 0:512], data0=rho2f[:, 0:512],
                                         data1=wE[0][:], initial=0.0,
                                         op0=MULT, op1=ADD)
            nc.vector.tensor_tensor_scan(out=g[:, 512:SH],
                                         data0=rho2f[:, 512:SH],
                                         data1=wE[1][:],
                                         initial=g[:, 511:512],
                                         op0=MULT, op1=ADD)

            # rotate-out (even positions only)
            p1 = work.tile([128, SH], BF16, tag="p1", name="p1")
            p2 = work.tile([128, SH], BF16, tag="p2", name="p2")
            nc.vector.tensor_tensor(out=p1[:], in0=g[:], in1=cosE[:], op=MULT)
            nc.vector.tensor_tensor(out=p2[:], in0=g[:], in1=sinE[:], op=MULT)

            # y even blocks: cm1@p1 + cm2@p2 + statD@obsT_even
            # y odd blocks:  cm1l@p1 + cm2l@p2 + cm1@u_odd + statD@obsT_odd
            # (pl, blk): pl 0=even (s=2m) half [0:64], 1=odd (s=2m+1) [64:128]
            py = [pY.tile([128, 512], F32, tag=f"pY{j}", name=f"pY{j}")
                  for j in range(2)]
            subs = [(pl, blk) for pl in range(2) for blk in range(2)]

            def sub_out(pl, blk):
                return py[blk][pl * 64:(pl + 1) * 64, :]

            uAc_odd = uAc[:].rearrange("p (n f) -> p f n", f=2)[:, 1]
            for pl, blk in subs:
                jl = slice(blk * 512, (blk + 1) * 512)
                nc.tensor.matmul(out=sub_out(pl, blk),
                                 lhsT=(cm1 if pl == 0 else cm1l),
                                 rhs=p1[:, jl], start=True, stop=False)
                nc.tensor.matmul(out=sub_out(pl, blk),
                                 lhsT=(cm2 if pl == 0 else cm2l),
                                 rhs=p2[:, jl], start=False, stop=False)
                if pl == 1:
                    nc.tensor.matmul(out=sub_out(pl, blk), lhsT=cm1,
                                     rhs=uAc_odd[:, jl], start=False,
                                     stop=False)
                for k in range(NDC):
                    base = obsT[k][:, blk * 1024:(blk + 1) * 1024]
                    obsP = base.rearrange("p (n f) -> p f n", f=2)[:, pl]
                    nc.tensor.matmul(out=sub_out(pl, blk), lhsT=statD[k],
                                     rhs=obsP, start=False,
                                     stop=(k == NDC - 1))

            ymax = small.tile([128, 2], F32, tag="ymax", name="ymax")
            for j in range(2):
                nc.vector.tensor_reduce(out=ymax[:, j:j + 1], in_=py[j][:],
                                        axis=X, op=MAX)
            nc.vector.tensor_reduce(out=lat128[:, b:b + 1], in_=ymax[:],
                                    axis=X, op=MAX)
            if b == 0:
                emit_xa()
            if b == NB // 2 - 1:
                emit_mlp_half(0)
            if b == NB - 1:
                emit_mlp_half(1)

        nc.sync.dma_start(out=out_d, in_=x3[:])

    nc.compile()
    return nc


_NC_CACHE = {}


def _get_nc():
    if "nc" not in _NC_CACHE:
        _NC_CACHE["nc"] = _build_nc()
    return _NC_CACHE["nc"]


def _host_tables(nu_log, theta_log, gamma_log, B_re, B_im, C_re, C_im, D,
                 W1, b1, W2, b2, W3, b3):
    f64 = np.float64
    bf = ml_dtypes.bfloat16
    rho_h = np.exp(-np.exp(nu_log.astype(f64)))          # [H]
    theta_h = np.exp(theta_log.astype(f64))              # [H]
    gamma_h = np.exp(gamma_log.astype(f64))              # [H]
    s = np.arange(S_, dtype=f64)
    phase = (theta_h[:, None] * s[None, :]) % (2 * np.pi)   # [H, S]
    cos_t = np.cos(phase)
    sin_t = np.sin(phase)

    def dup(x):  # [H,S] -> [128,S]
        return np.concatenate([x, x], axis=0)

    cosS = dup(cos_t).astype(bf)
    sinpm2 = np.concatenate([-sin_t, sin_t], axis=0).astype(bf)
    cosE = dup(cos_t)[:, 0::2].astype(bf)
    sinE = dup(sin_t)[:, 0::2].astype(bf)

    rho128 = np.concatenate([rho_h, rho_h]).astype(f64)     # [128]
    rho1 = rho128.astype(np.float32).reshape(128, 1)
    rho2f = np.broadcast_to((rho128 ** 2).astype(np.float32)[:, None],
                            (128, SH)).copy()

    Bg_re = (B_re.astype(f64) * gamma_h[:, None])        # [H, D_IN]
    Bg_im = (B_im.astype(f64) * gamma_h[:, None])
    statA = np.concatenate([Bg_re.T, Bg_im.T], axis=1)   # [D_IN, 128]
    statA = statA.reshape(NDC, 128, 128).astype(bf)
    permP = np.zeros((128, 128), dtype=bf)
    for m in range(128):
        permP[m ^ 64, m] = 1
    ident = np.eye(128, dtype=bf)
    statD = D.T.reshape(NDC, 128, D_OUT).astype(bf)

    cm1 = np.concatenate([C_re.T, -C_im.T], axis=0).astype(bf)
    cm2 = np.concatenate([-C_im.T, -C_re.T], axis=0).astype(bf)
    inC_re = C_re.astype(f64)
    inC_im = C_im.astype(f64)

    w1lat = np.zeros((128, 64), dtype=np.float64)
    w1lat[:H_] = W1[:, :H_].T
    w1lat = w1lat.astype(bf)                             # [128, 64] padded
    w1act = W1[:, H_:].T.reshape(NDC, 128, D_MLP).astype(bf)
    w2 = W2.T.astype(bf)                                 # [64, 32]
    w3 = W3.T.astype(bf)                                 # [32, 1]

    tabs = np.concatenate([cosS, sinpm2, cosE, sinE], axis=1)
    rhopk = np.concatenate([rho2f, rho1], axis=1).astype(np.float32)
    # statpk layout: statA(3x128) permP ident statD(3x64) cm1 cm2 w1lat w1act(3x64)
    # lambda-folded C: C' = C * diag(lambda)
    lam_re = (rho128[:H_] * np.cos(theta_h))
    lam_im = (rho128[:H_] * np.sin(theta_h))
    Cp_re = inC_re * lam_re[None, :] - inC_im * lam_im[None, :]
    Cp_im = inC_re * lam_im[None, :] + inC_im * lam_re[None, :]
    cm1l = np.concatenate([Cp_re.T, -Cp_im.T], axis=0).astype(bf)
    cm2l = np.concatenate([-Cp_im.T, -Cp_re.T], axis=0).astype(bf)
    rhoI = (np.eye(128) * rho128[None, :]).astype(bf)
    rhoP = (permP.astype(np.float64) * rho128[None, :]).astype(bf)
    statpk = np.concatenate(
        [np.concatenate([statA[k] for k in range(NDC)], axis=1),
         permP, ident,
         np.concatenate([statD[k] for k in range(NDC)], axis=1),
         cm1, cm2, w1lat,
         np.concatenate([w1act[k] for k in range(NDC)], axis=1),
         rhoI, rhoP, cm1l, cm2l],
        axis=1).astype(bf)
    assert statpk.shape == (128, 1600), statpk.shape
    return dict(
        tabs=tabs, rhopk=rhopk, statpk=statpk,
        w2=w2, w3=w3,
        b1=b1.reshape(64, 1).astype(np.float32),
        b2=b2.reshape(32, 1).astype(np.float32),
        b3=b3.reshape(1, 1).astype(np.float32),
    )


def kernel(observations, actions, nu_log, theta_log, gamma_log,
           B_re, B_im, C_re, C_im, D, W1, b1, W2, b2, W3, b3,
           _trace=False, _tmpdir=None):
    obs_bf = np.asarray(observations, dtype=np.float32).astype(
        ml_dtypes.bfloat16)
    act_bf = np.asarray(actions, dtype=np.float32).astype(ml_dtypes.bfloat16)
    # host-side transposes: obsT [B, NDC, 128, S]
    obsT_all = np.ascontiguousarray(obs_bf.transpose(0, 2, 1)).reshape(
        B_, NDC, 128, S_)
    tables = _host_tables(np.asarray(nu_log), np.asarray(theta_log),
                          np.asarray(gamma_log), np.asarray(B_re),
                          np.asarray(B_im), np.asarray(C_re),
                          np.asarray(C_im), np.asarray(D),
                          np.asarray(W1), np.asarray(b1), np.asarray(W2),
                          np.asarray(b2), np.asarray(W3), np.asarray(b3))
    in_maps = []
    for c in range(NCORES):
        m = dict(tables)
        m["obsT"] = np.ascontiguousarray(obsT_all[c * NB:(c + 1) * NB])
        act_c = act_bf[c * NB:(c + 1) * NB].reshape(NB * A_, D_IN)
        m["actT"] = np.ascontiguousarray(act_c.T).reshape(NDC, 128, NB * A_)
        in_maps.append(m)

    nc = _get_nc()
    res = run_bass_kernel_spmd(nc, in_maps, core_ids=list(range(NCORES)),
                               trace=_trace, tmpdir=_tmpdir)
    outs = []
    for c in range(NCORES):
        outs.append(np.asarray(res.results[c]["out"]).reshape(NB, A_, 1))
    full = np.concatenate(outs, axis=0).astype(np.float32)
    if _trace:
        return full, res
    return full


# revision 22
# speedup vs baseline: 1.0043x; 1.0043x over previous
"""Trainium2 Bass kernel for nn_ActionScoringModel (LRU + max-pool + tanh MLP).

Strategy: data-parallel over batch (64 = 8 cores x 8 batches). No collectives.
Per core (V2.1 pipeline):
  - obs/act cast to bf16 AND transposed on host -> obsT [NB, 3, 128, S],
    actT [3, 128, NB*A]; device does plain contiguous DMA loads only.
  - u = statA @ obsT, k-major stationary reuse (PSUM, 4 banks)
  - rotate-in: uAc = bf16 copy of u (Act); t1 = uAc (.) cos (DVE),
    t2 = uAc (.) sin' (gpsimd); v = I@t1 + P@t2 on PE (partition swap folded
    into permutation stationary P), Act copies v -> SBUF (padded by 1 col).
  - scan decimation x2: wE_m = rho v_{2m-1} + v_{2m} (stt on DVE);
    hardware scan of length 1024 with rho^2 (DVE); odd positions fixed up
    with one stt: gO = rho gE + v_odd. g layout = [even plane | odd plane]
    which is fine because latent = max over s (permutation invariant).
  - rotate-out: p1 = g (.) cos2P, p2 = g (.) sin2P (plane-ordered tables)
  - y = CM1@p1 + CM2@p2 + D@obsT(strided) per 512-block; two blocks share
    one PSUM bank (partitions 0:64 / 64:128) so each max-reduce covers two
    blocks; final cross-partition pair-max via P matmul + tensor MAX.
  - tanh MLP head on [latent, act].
"""

import sys
import numpy as np
from contextlib import ExitStack

for _p in ("/opt/trn_rl_repo",):
    if _p not in sys.path:
        sys.path.insert(0, _p)

import ml_dtypes
import concourse.bass as bass
import concourse.tile as tile
from concourse import bacc, mybir
from concourse.bass_utils import run_bass_kernel_spmd

BF16 = mybir.dt.bfloat16
F32 = mybir.dt.float32

B_, S_, A_, D_IN, H_, D_OUT, D_MLP = 64, 2048, 128, 384, 64, 64, 64
NCORES = 8
NB = B_ // NCORES          # 8 batches per core
NSB = S_ // 512            # 4 s-blocks of 512
NDC = D_IN // 128          # 3 d-chunks
SH = S_ // 2               # 1024, scan length / plane width


def _build_nc():
    nc = bacc.Bacc("TRN2", target_bir_lowering=False, debug=False,
                   num_devices=1)

    # ---- DRAM I/O ----
    obsT_d = nc.dram_tensor("obsT", [NB, NDC, 128, S_], BF16,
                            kind="ExternalInput").ap()
    actT_d = nc.dram_tensor("actT", [NDC, 128, NB * A_], BF16,
                            kind="ExternalInput").ap()
    tabs_d = nc.dram_tensor("tabs", [128, 3 * S_], BF16, kind="ExternalInput").ap()
    rhopk_d = nc.dram_tensor("rhopk", [128, SH + 1], F32, kind="ExternalInput").ap()
    statpk_d = nc.dram_tensor("statpk", [128, 1600], BF16, kind="ExternalInput").ap()
    w2_d = nc.dram_tensor("w2", [64, 32], BF16, kind="ExternalInput").ap()
    w3_d = nc.dram_tensor("w3", [32, 1], BF16, kind="ExternalInput").ap()
    b1_d = nc.dram_tensor("b1", [64, 1], F32, kind="ExternalInput").ap()
    b2_d = nc.dram_tensor("b2", [32, 1], F32, kind="ExternalInput").ap()
    b3_d = nc.dram_tensor("b3", [1, 1], F32, kind="ExternalInput").ap()
    out_d = nc.dram_tensor("out", [1, NB * A_], F32, kind="ExternalOutput").ap()

    MULT = mybir.AluOpType.mult
    ADD = mybir.AluOpType.add
    MAX = mybir.AluOpType.max
    TANH = mybir.ActivationFunctionType.Tanh
    X = mybir.AxisListType.X

    with tile.TileContext(nc) as tc, ExitStack() as ctx:
        const = ctx.enter_context(tc.tile_pool(name="const", bufs=1))
        obsT_pool = ctx.enter_context(tc.tile_pool(name="obsT", bufs=3))
        work = ctx.enter_context(tc.tile_pool(name="work", bufs=2))
        tpool = ctx.enter_context(tc.tile_pool(name="tpool", bufs=3))
        pUA = ctx.enter_context(tc.tile_pool(name="pUA", bufs=1, space="PSUM"))
        pWE = ctx.enter_context(tc.tile_pool(name="pWE", bufs=1, space="PSUM"))
        pY = ctx.enter_context(tc.tile_pool(name="pY", bufs=1, space="PSUM"))
        small = ctx.enter_context(tc.tile_pool(name="small", bufs=1))

        def load_const(ap_d, shape, dtype, suffix=""):
            nm = f"c_{ap_d.tensor.name}{suffix}"
            t = const.tile(shape, dtype, tag=nm, name=nm)
            nc.scalar.dma_start(out=t[:], in_=ap_d)
            return t

        # packed consts: stationaries first (small, unblock compute), then
        # big tables split across both hwdge queues
        statpk = const.tile([128, 1600], BF16, tag="statpk", name="statpk")
        nc.scalar.dma_start(out=statpk[:], in_=statpk_d)
        tabs = const.tile([128, 3 * S_], BF16, tag="tabs", name="tabs")
        nc.scalar.dma_start(out=tabs[:, 0:S_], in_=tabs_d[:, 0:S_])
        nc.scalar.dma_start(out=tabs[:, S_:2 * S_], in_=tabs_d[:, S_:2 * S_])
        rhopk = const.tile([128, SH + 1], F32, tag="rhopk", name="rhopk")
        w2 = load_const(w2_d, [64, 32], BF16)
        w3 = load_const(w3_d, [32, 1], BF16)
        b1 = load_const(b1_d, [64, 1], F32)
        b2 = load_const(b2_d, [32, 1], F32)
        b3 = load_const(b3_d, [1, 1], F32)

        cosS = tabs[:, 0:S_]
        sinpm2 = tabs[:, S_:2 * S_]
        cosE = tabs[:, 2 * S_:2 * S_ + SH]
        sinE = tabs[:, 2 * S_ + SH:3 * S_]
        rho2f = rhopk[:, 0:SH]
        rho1 = rhopk[:, SH:SH + 1]
        statA = [statpk[:, k * 128:(k + 1) * 128] for k in range(NDC)]
        permP = statpk[:, 384:512]
        ident = statpk[:, 512:640]
        statD = [statpk[:, 640 + k * 64:640 + (k + 1) * 64] for k in range(NDC)]
        cm1 = statpk[:, 832:896]
        cm2 = statpk[:, 896:960]
        w1lat = statpk[:, 960:1024]
        w1act = [statpk[:, 1024 + k * 64:1024 + (k + 1) * 64] for k in range(NDC)]
        rhoI = statpk[:, 1216:1344]
        rhoP = statpk[:, 1344:1472]
        cm1l = statpk[:, 1472:1536]
        cm2l = statpk[:, 1536:1600]

        lat128 = small.tile([128, NB], F32)     # per-pair latent maxima

        # prefetch batch-0 obsT ahead of everything else on the sync queue
        obsT0 = [obsT_pool.tile([128, S_], BF16, tag=f"obsT{k}",
                                name=f"obsT{k}") for k in range(NDC)]
        for k in range(NDC):
            nc.sync.dma_start(out=obsT0[k][:], in_=obsT_d[0, k])

        # action-side MLP input (independent of the LRU path): compute
        # xa = W1act @ actT early so the tail only needs activations
        actT = [small.tile([128, NB * A_], BF16, tag=f"actT{k}",
                           name=f"actT{k}") for k in range(NDC)]
        for k in range(NDC):
            nc.scalar.dma_start(out=actT[k][:], in_=actT_d[k])
        nc.scalar.dma_start(out=rhopk[:], in_=rhopk_d)
        nc.sync.dma_start(out=tabs[:, 2 * S_:], in_=tabs_d[:, 2 * S_:])
        xa = small.tile([64, NB * A_], F32, tag="xa", name="xa")
        lat128b = small.tile([128, NB], BF16)
        latf = small.tile([64, NB], F32)
        latb = small.tile([64, NB], BF16)
        latWb = small.tile([64, NB], F32)
        x1 = small.tile([64, NB * A_], BF16)
        x2 = small.tile([32, NB * A_], BF16)
        x3 = small.tile([1, NB * A_], F32)

        def emit_mlp_half(h):
            bs = slice(h * (NB // 2), (h + 1) * (NB // 2))
            hl = slice(h * 512, (h + 1) * 512)
            nc.vector.tensor_copy(out=lat128b[:, bs], in_=lat128[:, bs])
            pswap = pWE.tile([128, 512], F32, tag="wE0", name="pswap")
            nc.tensor.matmul(out=pswap[:, 0:NB // 2], lhsT=permP,
                             rhs=lat128b[:, bs], start=True, stop=True)
            nc.vector.tensor_tensor(out=latf[:, bs], in0=lat128[0:64, bs],
                                    in1=pswap[0:64, 0:NB // 2], op=MAX)
            nc.vector.tensor_copy(out=latb[:, bs], in_=latf[:, bs])
            platW = pWE.tile([128, 512], F32, tag="wE1", name="platW")
            nc.tensor.matmul(out=platW[:64, 0:NB // 2], lhsT=w1lat[0:64, :],
                             rhs=latb[:, bs], start=True, stop=True)
            nc.vector.tensor_scalar(out=latWb[:, bs],
                                    in0=platW[:64, 0:NB // 2],
                                    scalar1=b1[:], scalar2=None, op0=ADD)
            for bb in range(NB // 2):
                b_idx = h * 4 + bb
                nc.scalar.activation(
                    out=x1[:, b_idx * A_:(b_idx + 1) * A_],
                    in_=xa[:, b_idx * A_:(b_idx + 1) * A_],
                    func=TANH, bias=latWb[:, b_idx:b_idx + 1], scale=1.0)
            px = pWE.tile([128, 512], F32, tag="wE0", name="px2")
            nc.tensor.matmul(out=px[:32, :], lhsT=w2[:], rhs=x1[:, hl],
                             start=True, stop=True)
            nc.scalar.activation(out=x2[:, hl], in_=px[:32, :], func=TANH,
                                 bias=b2[:], scale=1.0)
            px3 = pWE.tile([128, 512], F32, tag="wE1", name="px3")
            nc.tensor.matmul(out=px3[:1, :], lhsT=w3[:], rhs=x2[:, hl],
                             start=True, stop=True)
            nc.scalar.activation(out=x3[:, hl], in_=px3[:1, :], func=TANH,
                                 bias=b3[:], scale=1.0)

        def emit_xa():
            for half in range(2):
                hl = slice(half * 512, (half + 1) * 512)
                pxa = pWE.tile([128, 512], F32, tag="wE0", name="pxa")
                for k in range(NDC):
                    nc.tensor.matmul(out=pxa[:64, :], lhsT=w1act[k],
                                     rhs=actT[k][:, hl],
                                     start=(k == 0), stop=(k == NDC - 1))
                nc.scalar.copy(out=xa[:, hl], in_=pxa[:64, :])

        # ---------------- main loop over local batches ----------------
        for b in range(NB):
            if b == 0:
                obsT = obsT0
            else:
                obsT = [obsT_pool.tile([128, S_], BF16, tag=f"obsT{k}",
                                       name=f"obsT{k}")
                        for k in range(NDC)]
                for k in range(NDC):
                    nc.sync.dma_start(out=obsT[k][:], in_=obsT_d[b, k])

            # u = statA @ obsT, k-major (3 weight loads per batch)
            uA = [None] * NSB
            for k in range(NDC):
                for i in range(NSB):
                    if k == 0:
                        uA[i] = pUA.tile([128, 512], F32, tag=f"uA{i}",
                                         name=f"uA{i}")
                    nc.tensor.matmul(
                        out=uA[i][:], lhsT=statA[k],
                        rhs=obsT[k][:, i * 512:(i + 1) * 512],
                        start=(k == 0), stop=(k == NDC - 1))

            # rotate-in into padded full-batch tiles (col0 = 0)
            t1 = work.tile([128, S_ + 1], BF16, tag="t1", name="t1")
            t2 = work.tile([128, S_ + 1], BF16, tag="t2", name="t2")
            nc.gpsimd.memset(t1[:, 0:1], 0.0)
            nc.gpsimd.memset(t2[:, 0:1], 0.0)
            uAc = work.tile([128, S_], BF16, tag="uAc", name="uAc")
            for i in range(NSB):
                sl = slice(i * 512, (i + 1) * 512)
                slp = slice(1 + i * 512, 1 + (i + 1) * 512)
                nc.scalar.copy(out=uAc[:, sl], in_=uA[i][:])
                nc.vector.tensor_tensor(out=t1[:, slp], in0=uA[i][:],
                                        in1=cosS[:, sl], op=MULT)
                nc.gpsimd.tensor_tensor(out=t2[:, slp], in0=uAc[:, sl],
                                        in1=sinpm2[:, sl], op=MULT)

            # wE = rhoI@t1_odd + rhoP@t2_odd + I@t1_even + P@t2_even on PE
            # (wE_m = rho*v_{2m-1} + v_{2m}, v = I@t1 + P@t2)
            t1_lo = t1[:, 0:S_].rearrange("p (n f) -> p f n", f=2)[:, 0]
            t1_hi = t1[:, 1:S_ + 1].rearrange("p (n f) -> p f n", f=2)[:, 0]
            t2_lo = t2[:, 0:S_].rearrange("p (n f) -> p f n", f=2)[:, 0]
            t2_hi = t2[:, 1:S_ + 1].rearrange("p (n f) -> p f n", f=2)[:, 0]
            wE = [pWE.tile([128, 512], F32, tag=f"wE{j}", name=f"wE{j}")
                  for j in range(2)]
            for j in range(2):
                jl = slice(j * 512, (j + 1) * 512)
                nc.tensor.matmul(out=wE[j][:], lhsT=rhoI, rhs=t1_lo[:, jl],
                                 start=True, stop=False)
                nc.tensor.matmul(out=wE[j][:], lhsT=rhoP, rhs=t2_lo[:, jl],
                                 start=False, stop=False)
                nc.tensor.matmul(out=wE[j][:], lhsT=ident, rhs=t1_hi[:, jl],
                                 start=False, stop=False)
                nc.tensor.matmul(out=wE[j][:], lhsT=permP, rhs=t2_hi[:, jl],
                                 start=False, stop=True)

            # chained length-512 scans with rho^2 over the wE PSUM banks
            g = work.tile([128, SH], BF16, tag="g", name="g")
            nc.vector.tensor_tensor_scan(out=g[:, 0:512], data0=rho2f[:, 0:512],
                                         data1=wE[0][:], initial=0.0,
                                         op0=MULT, op1=ADD)
            nc.vector.tensor_tensor_scan(out=g[:, 512:SH],
                                         data0=rho2f[:, 512:SH],
                                         data1=wE[1][:],
                                         initial=g[:, 511:512],
                                         op0=MULT, op1=ADD)

            # rotate-out (even positions only)
            p1 = work.tile([128, SH], BF16, tag="p1", name="p1")
            p2 = work.tile([128, SH], BF16, tag="p2", name="p2")
            nc.vector.tensor_tensor(out=p1[:], in0=g[:], in1=cosE[:], op=MULT)
            nc.vector.tensor_tensor(out=p2[:], in0=g[:], in1=sinE[:], op=MULT)

            # y even blocks: cm1@p1 + cm2@p2 + statD@obsT_even
            # y odd blocks:  cm1l@p1 + cm2l@p2 + cm1@u_odd + statD@obsT_odd
            # (pl, blk): pl 0=even (s=2m) half [0:64], 1=odd (s=2m+1) [64:128]
            py = [pY.tile([128, 512], F32, tag=f"pY{j}", name=f"pY{j}")
                  for j in range(2)]
            subs = [(pl, blk) for pl in range(2) for blk in range(2)]

            def sub_out(pl, blk):
                return py[blk][pl * 64:(pl + 1) * 64, :]

            uAc_odd = uAc[:].rearrange("p (n f) -> p f n", f=2)[:, 1]
            for pl, blk in subs:
                jl = slice(blk * 512, (blk + 1) * 512)
                nc.tensor.matmul(out=sub_out(pl, blk),
                                 lhsT=(cm1 if pl == 0 else cm1l),
                                 rhs=p1[:, jl], start=True, stop=False)
                nc.tensor.matmul(out=sub_out(pl, blk),
                                 lhsT=(cm2 if pl == 0 else cm2l),
                                 rhs=p2[:, jl], start=False, stop=False)
                if pl == 1:
                    nc.tensor.matmul(out=sub_out(pl, blk), lhsT=cm1,
                                     rhs=uAc_odd[:, jl], start=False,
                                     stop=False)
                for k in range(NDC):
                    base = obsT[k][:, blk * 1024:(blk + 1) * 1024]
                    obsP = base.rearrange("p (n f) -> p f n", f=2)[:, pl]
                    nc.tensor.matmul(out=sub_out(pl, blk), lhsT=statD[k],
                                     rhs=obsP, start=False,
                                     stop=(k == NDC - 1))

            ymax = small.tile([128, 2], F32, tag="ymax", name="ymax")
            for j in range(2):
                nc.vector.tensor_reduce(out=ymax[:, j:j + 1], in_=py[j][:],
                                        axis=X, op=MAX)
            nc.vector.tensor_reduce(out=lat128[:, b:b + 1], in_=ymax[:],
                                    axis=X, op=MAX)
            if b == 0:
                emit_xa()
            if b == NB // 2 - 1:
                emit_mlp_half(0)
            if b == NB - 1:
                emit_mlp_half(1)

        nc.sync.dma_start(out=out_d, in_=x3[:])

    nc.compile()
    return nc


_NC_CACHE = {}


def _get_nc():
    if "nc" not in _NC_CACHE:
        _NC_CACHE["nc"] = _build_nc()
    return _NC_CACHE["nc"]


def _host_tables(nu_log, theta_log, gamma_log, B_re, B_im, C_re, C_im, D,
                 W1, b1, W2, b2, W3, b3):
    f64 = np.float64
    bf = ml_dtypes.bfloat16
    rho_h = np.exp(-np.exp(nu_log.astype(f64)))          # [H]
    theta_h = np.exp(theta_log.astype(f64))              # [H]
    gamma_h = np.exp(gamma_log.astype(f64))              # [H]
    s = np.arange(S_, dtype=f64)
    phase = (theta_h[:, None] * s[None, :]) % (2 * np.pi)   # [H, S]
    cos_t = np.cos(phase)
    sin_t = np.sin(phase)

    def dup(x):  # [H,S] -> [128,S]
        return np.concatenate([x, x], axis=0)

    cosS = dup(cos_t).astype(bf)
    sinpm2 = np.concatenate([-sin_t, sin_t], axis=0).astype(bf)
    cosE = dup(cos_t)[:, 0::2].astype(bf)
    sinE = dup(sin_t)[:, 0::2].astype(bf)

    rho128 = np.concatenate([rho_h, rho_h]).astype(f64)     # [128]
    rho1 = rho128.astype(np.float32).reshape(128, 1)
    rho2f = np.broadcast_to((rho128 ** 2).astype(np.float32)[:, None],
                            (128, SH)).copy()

    Bg_re = (B_re.astype(f64) * gamma_h[:, None])        # [H, D_IN]
    Bg_im = (B_im.astype(f64) * gamma_h[:, None])
    statA = np.concatenate([Bg_re.T, Bg_im.T], axis=1)   # [D_IN, 128]
    statA = statA.reshape(NDC, 128, 128).astype(bf)
    permP = np.zeros((128, 128), dtype=bf)
    for m in range(128):
        permP[m ^ 64, m] = 1
    ident = np.eye(128, dtype=bf)
    statD = D.T.reshape(NDC, 128, D_OUT).astype(bf)

    cm1 = np.concatenate([C_re.T, -C_im.T], axis=0).astype(bf)
    cm2 = np.concatenate([-C_im.T, -C_re.T], axis=0).astype(bf)
    inC_re = C_re.astype(f64)
    inC_im = C_im.astype(f64)

    w1lat = np.zeros((128, 64), dtype=np.float64)
    w1lat[:H_] = W1[:, :H_].T
    w1lat = w1lat.astype(bf)                             # [128, 64] padded
    w1act = W1[:, H_:].T.reshape(NDC, 128, D_MLP).astype(bf)
    w2 = W2.T.astype(bf)                                 # [64, 32]
    w3 = W3.T.astype(bf)                                 # [32, 1]

    tabs = np.concatenate([cosS, sinpm2, cosE, sinE], axis=1)
    rhopk = np.concatenate([rho2f, rho1], axis=1).astype(np.float32)
    # statpk layout: statA(3x128) permP ident statD(3x64) cm1 cm2 w1lat w1act(3x64)
    # lambda-folded C: C' = C * diag(lambda)
    lam_re = (rho128[:H_] * np.cos(theta_h))
    lam_im = (rho128[:H_] * np.sin(theta_h))
    Cp_re = inC_re * lam_re[None, :] - inC_im * lam_im[None, :]
    Cp_im = inC_re * lam_im[None, :] + inC_im * lam_re[None, :]
    cm1l = np.concatenate([Cp_re.T, -Cp_im.T], axis=0).astype(bf)
    cm2l = np.concatenate([-Cp_im.T, -Cp_re.T], axis=0).astype(bf)
    rhoI = (np.eye(128) * rho128[None, :]).astype(bf)
    rhoP = (permP.astype(np.float64) * rho128[None, :]).astype(bf)
    statpk = np.concatenate(
        [np.concatenate([statA[k] for k in range(NDC)], axis=1),
         permP, ident,
         np.concatenate([statD[k] for k in range(NDC)], axis=1),
         cm1, cm2, w1lat,
         np.concatenate([w1act[k] for k in range(NDC)], axis=1),
         rhoI, rhoP, cm1l, cm2l],
        axis=1).astype(bf)
    assert statpk.shape == (128, 1600), statpk.shape
    return dict(
        tabs=tabs, rhopk=rhopk, statpk=statpk,
        w2=w2, w3=w3,
        b1=b1.reshape(64, 1).astype(np.float32),
        b2=b2.reshape(32, 1).astype(np.float32),
        b3=b3.reshape(1, 1).astype(np.float32),
    )


def kernel(observations, actions, nu_log, theta_log, gamma_log,
           B_re, B_im, C_re, C_im, D, W1, b1, W2, b2, W3, b3,
           _trace=False, _tmpdir=None):
    obs_bf = np.asarray(observations, dtype=np.float32).astype(
        ml_dtypes.bfloat16)
    act_bf = np.asarray(actions, dtype=np.float32).astype(ml_dtypes.bfloat16)
    # host-side transposes: obsT [B, NDC, 128, S]
    obsT_all = np.ascontiguousarray(obs_bf.transpose(0, 2, 1)).reshape(
        B_, NDC, 128, S_)
    tables = _host_tables(np.asarray(nu_log), np.asarray(theta_log),
                          np.asarray(gamma_log), np.asarray(B_re),
                          np.asarray(B_im), np.asarray(C_re),
                          np.asarray(C_im), np.asarray(D),
                          np.asarray(W1), np.asarray(b1), np.asarray(W2),
                          np.asarray(b2), np.asarray(W3), np.asarray(b3))
    in_maps = []
    for c in range(NCORES):
        m = dict(tables)
        m["obsT"] = np.ascontiguousarray(obsT_all[c * NB:(c + 1) * NB])
        act_c = act_bf[c * NB:(c + 1) * NB].reshape(NB * A_, D_IN)
        m["actT"] = np.ascontiguousarray(act_c.T).reshape(NDC, 128, NB * A_)
        in_maps.append(m)

    nc = _get_nc()
    res = run_bass_kernel_spmd(nc, in_maps, core_ids=list(range(NCORES)),
                               trace=_trace, tmpdir=_tmpdir)
    outs = []
    for c in range(NCORES):
        outs.append(np.asarray(res.results[c]["out"]).reshape(NB, A_, 1))
    full = np.concatenate(outs, axis=0).astype(np.float32)
    if _trace:
        return full, res
    return full


# revision 23
# speedup vs baseline: 1.0175x; 1.0132x over previous
"""Trainium2 Bass kernel for nn_ActionScoringModel (LRU + max-pool + tanh MLP).

Strategy: data-parallel over batch (64 = 8 cores x 8 batches). No collectives.
Per core (V2.1 pipeline):
  - obs/act cast to bf16 AND transposed on host -> obsT [NB, 3, 128, S],
    actT [3, 128, NB*A]; device does plain contiguous DMA loads only.
  - u = statA @ obsT, k-major stationary reuse (PSUM, 4 banks)
  - rotate-in: uAc = bf16 copy of u (Act); t1 = uAc (.) cos (DVE),
    t2 = uAc (.) sin' (gpsimd); v = I@t1 + P@t2 on PE (partition swap folded
    into permutation stationary P), Act copies v -> SBUF (padded by 1 col).
  - scan decimation x2: wE_m = rho v_{2m-1} + v_{2m} (stt on DVE);
    hardware scan of length 1024 with rho^2 (DVE); odd positions fixed up
    with one stt: gO = rho gE + v_odd. g layout = [even plane | odd plane]
    which is fine because latent = max over s (permutation invariant).
  - rotate-out: p1 = g (.) cos2P, p2 = g (.) sin2P (plane-ordered tables)
  - y = CM1@p1 + CM2@p2 + D@obsT(strided) per 512-block; two blocks share
    one PSUM bank (partitions 0:64 / 64:128) so each max-reduce covers two
    blocks; final cross-partition pair-max via P matmul + tensor MAX.
  - tanh MLP head on [latent, act].
"""

import sys
import numpy as np
from contextlib import ExitStack

for _p in ("/opt/trn_rl_repo",):
    if _p not in sys.path:
        sys.path.insert(0, _p)

import ml_dtypes
import concourse.bass as bass
import concourse.tile as tile
from concourse import bacc, mybir
from concourse.bass_utils import run_bass_kernel_spmd

BF16 = mybir.dt.bfloat16
F32 = mybir.dt.float32

B_, S_, A_, D_IN, H_, D_OUT, D_MLP = 64, 2048, 128, 384, 64, 64, 64
NCORES = 8
NB = B_ // NCORES          # 8 batches per core
NSB = S_ // 512            # 4 s-blocks of 512
NDC = D_IN // 128          # 3 d-chunks
SH = S_ // 2               # 1024, scan length / plane width


def _build_nc():
    nc = bacc.Bacc("TRN2", target_bir_lowering=False, debug=False,
                   num_devices=1)

    # ---- DRAM I/O ----
    obsT_d = nc.dram_tensor("obsT", [NB, NDC, 128, S_], BF16,
                            kind="ExternalInput").ap()
    actT_d = nc.dram_tensor("actT", [NDC, 128, NB * A_], BF16,
                            kind="ExternalInput").ap()
    tabs_d = nc.dram_tensor("tabs", [128, 3 * S_], BF16, kind="ExternalInput").ap()
    rhopk_d = nc.dram_tensor("rhopk", [128, SH + 1], F32, kind="ExternalInput").ap()
    statpk_d = nc.dram_tensor("statpk", [128, 1600], BF16, kind="ExternalInput").ap()
    w2_d = nc.dram_tensor("w2", [64, 32], BF16, kind="ExternalInput").ap()
    w3_d = nc.dram_tensor("w3", [32, 1], BF16, kind="ExternalInput").ap()
    b1_d = nc.dram_tensor("b1", [64, 1], F32, kind="ExternalInput").ap()
    b2_d = nc.dram_tensor("b2", [32, 1], F32, kind="ExternalInput").ap()
    b3_d = nc.dram_tensor("b3", [1, 1], F32, kind="ExternalInput").ap()
    out_d = nc.dram_tensor("out", [1, NB * A_], F32, kind="ExternalOutput").ap()

    MULT = mybir.AluOpType.mult
    ADD = mybir.AluOpType.add
    MAX = mybir.AluOpType.max
    TANH = mybir.ActivationFunctionType.Tanh
    X = mybir.AxisListType.X

    with tile.TileContext(nc) as tc, ExitStack() as ctx:
        const = ctx.enter_context(tc.tile_pool(name="const", bufs=1))
        obsT_pool = ctx.enter_context(tc.tile_pool(name="obsT", bufs=3))
        work = ctx.enter_context(tc.tile_pool(name="work", bufs=2))
        tpool = ctx.enter_context(tc.tile_pool(name="tpool", bufs=3))
        pUA = ctx.enter_context(tc.tile_pool(name="pUA", bufs=1, space="PSUM"))
        pWE = ctx.enter_context(tc.tile_pool(name="pWE", bufs=1, space="PSUM"))
        pY = ctx.enter_context(tc.tile_pool(name="pY", bufs=1, space="PSUM"))
        small = ctx.enter_context(tc.tile_pool(name="small", bufs=1))

        def load_const(ap_d, shape, dtype, suffix=""):
            nm = f"c_{ap_d.tensor.name}{suffix}"
            t = const.tile(shape, dtype, tag=nm, name=nm)
            nc.scalar.dma_start(out=t[:], in_=ap_d)
            return t

        # packed consts: stationaries first (small, unblock compute), then
        # big tables split across both hwdge queues
        statpk = const.tile([128, 1600], BF16, tag="statpk", name="statpk")
        nc.scalar.dma_start(out=statpk[:], in_=statpk_d)
        tabs = const.tile([128, 3 * S_], BF16, tag="tabs", name="tabs")
        nc.scalar.dma_start(out=tabs[:, 0:S_], in_=tabs_d[:, 0:S_])
        nc.scalar.dma_start(out=tabs[:, S_:2 * S_], in_=tabs_d[:, S_:2 * S_])
        rhopk = const.tile([128, SH + 1], F32, tag="rhopk", name="rhopk")
        w2 = load_const(w2_d, [64, 32], BF16)
        w3 = load_const(w3_d, [32, 1], BF16)
        b1 = load_const(b1_d, [64, 1], F32)
        b2 = load_const(b2_d, [32, 1], F32)
        b3 = load_const(b3_d, [1, 1], F32)

        cosS = tabs[:, 0:S_]
        sinpm2 = tabs[:, S_:2 * S_]
        cosE = tabs[:, 2 * S_:2 * S_ + SH]
        sinE = tabs[:, 2 * S_ + SH:3 * S_]
        rho2f = rhopk[:, 0:SH]
        rho1 = rhopk[:, SH:SH + 1]
        statA = [statpk[:, k * 128:(k + 1) * 128] for k in range(NDC)]
        permP = statpk[:, 384:512]
        ident = statpk[:, 512:640]
        statD = [statpk[:, 640 + k * 64:640 + (k + 1) * 64] for k in range(NDC)]
        cm1 = statpk[:, 832:896]
        cm2 = statpk[:, 896:960]
        w1lat = statpk[:, 960:1024]
        w1act = [statpk[:, 1024 + k * 64:1024 + (k + 1) * 64] for k in range(NDC)]
        rhoI = statpk[:, 1216:1344]
        rhoP = statpk[:, 1344:1472]
        cm1l = statpk[:, 1472:1536]
        cm2l = statpk[:, 1536:1600]

        lat128 = small.tile([128, NB], F32)     # per-pair latent maxima

        # prefetch batch-0 obsT ahead of everything else on the sync queue
        obsT0 = [obsT_pool.tile([128, S_], BF16, tag=f"obsT{k}",
                                name=f"obsT{k}") for k in range(NDC)]
        for k in range(NDC):
            nc.sync.dma_start(out=obsT0[k][:], in_=obsT_d[0, k])

        # action-side MLP input (independent of the LRU path): compute
        # xa = W1act @ actT early so the tail only needs activations
        actT = [small.tile([128, NB * A_], BF16, tag=f"actT{k}",
                           name=f"actT{k}") for k in range(NDC)]
        for k in range(NDC):
            nc.scalar.dma_start(out=actT[k][:], in_=actT_d[k])
        nc.scalar.dma_start(out=rhopk[:], in_=rhopk_d)
        nc.sync.dma_start(out=tabs[:, 2 * S_:], in_=tabs_d[:, 2 * S_:])
        xa = small.tile([64, NB * A_], F32, tag="xa", name="xa")
        lat128b = small.tile([128, NB], BF16)
        latf = small.tile([64, NB], F32)
        latb = small.tile([64, NB], BF16)
        latWb = small.tile([64, NB], F32)
        x1 = small.tile([64, NB * A_], BF16)
        x2 = small.tile([32, NB * A_], BF16)
        x3 = small.tile([1, NB * A_], F32)

        def emit_mlp_half(h):
            bs = slice(h * (NB // 2), (h + 1) * (NB // 2))
            hl = slice(h * 512, (h + 1) * 512)
            nc.vector.tensor_copy(out=lat128b[:, bs], in_=lat128[:, bs])
            pswap = pWE.tile([128, 512], F32, tag="wE0", name="pswap")
            nc.tensor.matmul(out=pswap[:, 0:NB // 2], lhsT=permP,
                             rhs=lat128b[:, bs], start=True, stop=True)
            nc.vector.tensor_tensor(out=latf[:, bs], in0=lat128[0:64, bs],
                                    in1=pswap[0:64, 0:NB // 2], op=MAX)
            nc.vector.tensor_copy(out=latb[:, bs], in_=latf[:, bs])
            platW = pWE.tile([128, 512], F32, tag="wE1", name="platW")
            nc.tensor.matmul(out=platW[:64, 0:NB // 2], lhsT=w1lat[0:64, :],
                             rhs=latb[:, bs], start=True, stop=True)
            nc.vector.tensor_scalar(out=latWb[:, bs],
                                    in0=platW[:64, 0:NB // 2],
                                    scalar1=b1[:], scalar2=None, op0=ADD)
            for bb in range(NB // 2):
                b_idx = h * 4 + bb
                nc.scalar.activation(
                    out=x1[:, b_idx * A_:(b_idx + 1) * A_],
                    in_=xa[:, b_idx * A_:(b_idx + 1) * A_],
                    func=TANH, bias=latWb[:, b_idx:b_idx + 1], scale=1.0)
            px = pWE.tile([128, 512], F32, tag="wE0", name="px2")
            nc.tensor.matmul(out=px[:32, :], lhsT=w2[:], rhs=x1[:, hl],
                             start=True, stop=True)
            nc.scalar.activation(out=x2[:, hl], in_=px[:32, :], func=TANH,
                                 bias=b2[:], scale=1.0)
            px3 = pWE.tile([128, 512], F32, tag="wE1", name="px3")
            nc.tensor.matmul(out=px3[:1, :], lhsT=w3[:], rhs=x2[:, hl],
                             start=True, stop=True)
            nc.scalar.activation(out=x3[:, hl], in_=px3[:1, :], func=TANH,
                                 bias=b3[:], scale=1.0)

        def emit_xa():
            for half in range(2):
                hl = slice(half * 512, (half + 1) * 512)
                pxa = pWE.tile([128, 512], F32, tag="wE0", name="pxa")
                for k in range(NDC):
                    nc.tensor.matmul(out=pxa[:64, :], lhsT=w1act[k],
                                     rhs=actT[k][:, hl],
                                     start=(k == 0), stop=(k == NDC - 1))
                nc.scalar.copy(out=xa[:, hl], in_=pxa[:64, :])

        # ------- software-pipelined main loop: statA(b) | wE(b-1) | y(b-2)
        st = {}

        def stage_load(b):
            if b == 0:
                st[b] = {"obsT": obsT0}
                return
            obsT = [obsT_pool.tile([128, S_], BF16, tag=f"obsT{k}",
                                   name=f"obsT{k}") for k in range(NDC)]
            for k in range(NDC):
                nc.sync.dma_start(out=obsT[k][:], in_=obsT_d[b, k])
            st[b] = {"obsT": obsT}

        def stage_statA(b):
            s = st[b]
            uA = [None] * NSB
            for k in range(NDC):
                for i in range(NSB):
                    if k == 0:
                        uA[i] = pUA.tile([128, 512], F32, tag=f"uA{i}",
                                         name=f"uA{i}")
                    nc.tensor.matmul(
                        out=uA[i][:], lhsT=statA[k],
                        rhs=s["obsT"][k][:, i * 512:(i + 1) * 512],
                        start=(k == 0), stop=(k == NDC - 1))
            s["uA"] = uA

        def stage_rot(b):
            s = st[b]
            t1 = work.tile([128, S_ + 1], BF16, tag="t1", name="t1")
            t2 = work.tile([128, S_ + 1], BF16, tag="t2", name="t2")
            nc.gpsimd.memset(t1[:, 0:1], 0.0)
            nc.gpsimd.memset(t2[:, 0:1], 0.0)
            uAc = work.tile([128, S_], BF16, tag="uAc", name="uAc")
            for i in range(NSB):
                sl = slice(i * 512, (i + 1) * 512)
                slp = slice(1 + i * 512, 1 + (i + 1) * 512)
                nc.scalar.copy(out=uAc[:, sl], in_=s["uA"][i][:])
                nc.vector.tensor_tensor(out=t1[:, slp], in0=s["uA"][i][:],
                                        in1=cosS[:, sl], op=MULT)
                nc.gpsimd.tensor_tensor(out=t2[:, slp], in0=uAc[:, sl],
                                        in1=sinpm2[:, sl], op=MULT)
            s["t1"], s["t2"], s["uAc"] = t1, t2, uAc

        def stage_wE(b):
            s = st[b]
            t1, t2 = s["t1"], s["t2"]
            t1_lo = t1[:, 0:S_].rearrange("p (n f) -> p f n", f=2)[:, 0]
            t1_hi = t1[:, 1:S_ + 1].rearrange("p (n f) -> p f n", f=2)[:, 0]
            t2_lo = t2[:, 0:S_].rearrange("p (n f) -> p f n", f=2)[:, 0]
            t2_hi = t2[:, 1:S_ + 1].rearrange("p (n f) -> p f n", f=2)[:, 0]
            wE = [pWE.tile([128, 512], F32, tag=f"wE{j}", name=f"wE{j}")
                  for j in range(2)]
            for j in range(2):
                jl = slice(j * 512, (j + 1) * 512)
                nc.tensor.matmul(out=wE[j][:], lhsT=rhoI, rhs=t1_lo[:, jl],
                                 start=True, stop=False)
                nc.tensor.matmul(out=wE[j][:], lhsT=rhoP, rhs=t2_lo[:, jl],
                                 start=False, stop=False)
                nc.tensor.matmul(out=wE[j][:], lhsT=ident, rhs=t1_hi[:, jl],
                                 start=False, stop=False)
                nc.tensor.matmul(out=wE[j][:], lhsT=permP, rhs=t2_hi[:, jl],
                                 start=False, stop=True)
            s["wE"] = wE

        def stage_scan_p(b):
            s = st[b]
            g = work.tile([128, SH], BF16, tag="g", name="g")
            nc.vector.tensor_tensor_scan(out=g[:, 0:512],
                                         data0=rho2f[:, 0:512],
                                         data1=s["wE"][0][:], initial=0.0,
                                         op0=MULT, op1=ADD)
            nc.vector.tensor_tensor_scan(out=g[:, 512:SH],
                                         data0=rho2f[:, 512:SH],
                                         data1=s["wE"][1][:],
                                         initial=g[:, 511:512],
                                         op0=MULT, op1=ADD)
            p1 = work.tile([128, SH], BF16, tag="p1", name="p1")
            p2 = work.tile([128, SH], BF16, tag="p2", name="p2")
            nc.vector.tensor_tensor(out=p1[:], in0=g[:], in1=cosE[:], op=MULT)
            nc.vector.tensor_tensor(out=p2[:], in0=g[:], in1=sinE[:], op=MULT)
            s["p1"], s["p2"] = p1, p2

        def stage_y(b):
            s = st[b]
            py = [pY.tile([128, 512], F32, tag=f"pY{j}", name=f"pY{j}")
                  for j in range(2)]
            subs = [(pl, blk) for pl in range(2) for blk in range(2)]

            def sub_out(pl, blk):
                return py[blk][pl * 64:(pl + 1) * 64, :]

            uAc_odd = s["uAc"][:].rearrange("p (n f) -> p f n", f=2)[:, 1]
            for pl, blk in subs:
                jl = slice(blk * 512, (blk + 1) * 512)
                nc.tensor.matmul(out=sub_out(pl, blk),
                                 lhsT=(cm1 if pl == 0 else cm1l),
                                 rhs=s["p1"][:, jl], start=True, stop=False)
                nc.tensor.matmul(out=sub_out(pl, blk),
                                 lhsT=(cm2 if pl == 0 else cm2l),
                                 rhs=s["p2"][:, jl], start=False, stop=False)
                if pl == 1:
                    nc.tensor.matmul(out=sub_out(pl, blk), lhsT=cm1,
                                     rhs=uAc_odd[:, jl], start=False,
                                     stop=False)
                for k in range(NDC):
                    base = s["obsT"][k][:, blk * 1024:(blk + 1) * 1024]
                    obsP = base.rearrange("p (n f) -> p f n", f=2)[:, pl]
                    nc.tensor.matmul(out=sub_out(pl, blk), lhsT=statD[k],
                                     rhs=obsP, start=False,
                                     stop=(k == NDC - 1))
            s["py"] = py

        def stage_red(b):
            s = st[b]
            ymax = small.tile([128, 2], F32, tag="ymax", name="ymax")
            for j in range(2):
                nc.vector.tensor_reduce(out=ymax[:, j:j + 1],
                                        in_=s["py"][j][:], axis=X, op=MAX)
            nc.vector.tensor_reduce(out=lat128[:, b:b + 1], in_=ymax[:],
                                    axis=X, op=MAX)
            del st[b]

        for i in range(NB + 2):
            if i < NB:
                stage_load(i)
                stage_statA(i)
                stage_rot(i)
            if 1 <= i <= NB:
                stage_wE(i - 1)
                stage_scan_p(i - 1)
            if i >= 2:
                stage_y(i - 2)
                stage_red(i - 2)
            if i == 0:
                emit_xa()
            if i == NB // 2 + 1:
                emit_mlp_half(0)
        emit_mlp_half(1)

        nc.sync.dma_start(out=out_d, in_=x3[:])

    nc.compile()
    return nc


_NC_CACHE = {}


def _get_nc():
    if "nc" not in _NC_CACHE:
        _NC_CACHE["nc"] = _build_nc()
    return _NC_CACHE["nc"]


def _host_tables(nu_log, theta_log, gamma_log, B_re, B_im, C_re, C_im, D,
                 W1, b1, W2, b2, W3, b3):
    f64 = np.float64
    bf = ml_dtypes.bfloat16
    rho_h = np.exp(-np.exp(nu_log.astype(f64)))          # [H]
    theta_h = np.exp(theta_log.astype(f64))              # [H]
    gamma_h = np.exp(gamma_log.astype(f64))              # [H]
    s = np.arange(S_, dtype=f64)
    phase = (theta_h[:, None] * s[None, :]) % (2 * np.pi)   # [H, S]
    cos_t = np.cos(phase)
    sin_t = np.sin(phase)

    def dup(x):  # [H,S] -> [128,S]
        return np.concatenate([x, x], axis=0)

    cosS = dup(cos_t).astype(bf)
    sinpm2 = np.concatenate([-sin_t, sin_t], axis=0).astype(bf)
    cosE = dup(cos_t)[:, 0::2].astype(bf)
    sinE = dup(sin_t)[:, 0::2].astype(bf)

    rho128 = np.concatenate([rho_h, rho_h]).astype(f64)     # [128]
    rho1 = rho128.astype(np.float32).reshape(128, 1)
    rho2f = np.broadcast_to((rho128 ** 2).astype(np.float32)[:, None],
                            (128, SH)).copy()

    Bg_re = (B_re.astype(f64) * gamma_h[:, None])        # [H, D_IN]
    Bg_im = (B_im.astype(f64) * gamma_h[:, None])
    statA = np.concatenate([Bg_re.T, Bg_im.T], axis=1)   # [D_IN, 128]
    statA = statA.reshape(NDC, 128, 128).astype(bf)
    permP = np.zeros((128, 128), dtype=bf)
    for m in range(128):
        permP[m ^ 64, m] = 1
    ident = np.eye(128, dtype=bf)
    statD = D.T.reshape(NDC, 128, D_OUT).astype(bf)

    cm1 = np.concatenate([C_re.T, -C_im.T], axis=0).astype(bf)
    cm2 = np.concatenate([-C_im.T, -C_re.T], axis=0).astype(bf)
    inC_re = C_re.astype(f64)
    inC_im = C_im.astype(f64)

    w1lat = np.zeros((128, 64), dtype=np.float64)
    w1lat[:H_] = W1[:, :H_].T
    w1lat = w1lat.astype(bf)                             # [128, 64] padded
    w1act = W1[:, H_:].T.reshape(NDC, 128, D_MLP).astype(bf)
    w2 = W2.T.astype(bf)                                 # [64, 32]
    w3 = W3.T.astype(bf)                                 # [32, 1]

    tabs = np.concatenate([cosS, sinpm2, cosE, sinE], axis=1)
    rhopk = np.concatenate([rho2f, rho1], axis=1).astype(np.float32)
    # statpk layout: statA(3x128) permP ident statD(3x64) cm1 cm2 w1lat w1act(3x64)
    # lambda-folded C: C' = C * diag(lambda)
    lam_re = (rho128[:H_] * np.cos(theta_h))
    lam_im = (rho128[:H_] * np.sin(theta_h))
    Cp_re = inC_re * lam_re[None, :] - inC_im * lam_im[None, :]
    Cp_im = inC_re * lam_im[None, :] + inC_im * lam_re[None, :]
    cm1l = np.concatenate([Cp_re.T, -Cp_im.T], axis=0).astype(bf)
    cm2l = np.concatenate([-Cp_im.T, -Cp_re.T], axis=0).astype(bf)
    rhoI = (np.eye(128) * rho128[None, :]).astype(bf)
    rhoP = (permP.astype(np.float64) * rho128[None, :]).astype(bf)
    statpk = np.concatenate(
        [np.concatenate([statA[k] for k in range(NDC)], axis=1),
         permP, ident,
         np.concatenate([statD[k] for k in range(NDC)], axis=1),
         cm1, cm2, w1lat,
         np.concatenate([w1act[k] for k in range(NDC)], axis=1),
         rhoI, rhoP, cm1l, cm2l],
        axis=1).astype(bf)
    assert statpk.shape == (128, 1600), statpk.shape
    return dict(
        tabs=tabs, rhopk=rhopk, statpk=statpk,
        w2=w2, w3=w3,
        b1=b1.reshape(64, 1).astype(np.float32),
        b2=b2.reshape(32, 1).astype(np.float32),
        b3=b3.reshape(1, 1).astype(np.float32),
    )


def kernel(observations, actions, nu_log, theta_log, gamma_log,
           B_re, B_im, C_re, C_im, D, W1, b1, W2, b2, W3, b3,
           _trace=False, _tmpdir=None):
    obs_bf = np.asarray(observations, dtype=np.float32).astype(
        ml_dtypes.bfloat16)
    act_bf = np.asarray(actions, dtype=np.float32).astype(ml_dtypes.bfloat16)
    # host-side transposes: obsT [B, NDC, 128, S]
    obsT_all = np.ascontiguousarray(obs_bf.transpose(0, 2, 1)).reshape(
        B_, NDC, 128, S_)
    tables = _host_tables(np.asarray(nu_log), np.asarray(theta_log),
                          np.asarray(gamma_log), np.asarray(B_re),
                          np.asarray(B_im), np.asarray(C_re),
                          np.asarray(C_im), np.asarray(D),
                          np.asarray(W1), np.asarray(b1), np.asarray(W2),
                          np.asarray(b2), np.asarray(W3), np.asarray(b3))
    in_maps = []
    for c in range(NCORES):
        m = dict(tables)
        m["obsT"] = np.ascontiguousarray(obsT_all[c * NB:(c + 1) * NB])
        act_c = act_bf[c * NB:(c + 1) * NB].reshape(NB * A_, D_IN)
        m["actT"] = np.ascontiguousarray(act_c.T).reshape(NDC, 128, NB * A_)
        in_maps.append(m)

    nc = _get_nc()
    res = run_bass_kernel_spmd(nc, in_maps, core_ids=list(range(NCORES)),
                               trace=_trace, tmpdir=_tmpdir)
    outs = []
    for c in range(NCORES):
        outs.append(np.asarray(res.results[c]["out"]).reshape(NB, A_, 1))
    full = np.concatenate(outs, axis=0).astype(np.float32)
    if _trace:
        return full, res
    return full


# revision 24
# speedup vs baseline: 1.1446x; 1.1249x over previous
"""Trainium2 Bass kernel for nn_ActionScoringModel (LRU + max-pool + tanh MLP).

Strategy: data-parallel over batch (64 = 8 cores x 8 batches). No collectives.
Per core (V2.1 pipeline):
  - obs/act cast to bf16 AND transposed on host -> obsT [NB, 3, 128, S],
    actT [3, 128, NB*A]; device does plain contiguous DMA loads only.
  - u = statA @ obsT, k-major stationary reuse (PSUM, 4 banks)
  - rotate-in: uAc = bf16 copy of u (Act); t1 = uAc (.) cos (DVE),
    t2 = uAc (.) sin' (gpsimd); v = I@t1 + P@t2 on PE (partition swap folded
    into permutation stationary P), Act copies v -> SBUF (padded by 1 col).
  - scan decimation x2: wE_m = rho v_{2m-1} + v_{2m} (stt on DVE);
    hardware scan of length 1024 with rho^2 (DVE); odd positions fixed up
    with one stt: gO = rho gE + v_odd. g layout = [even plane | odd plane]
    which is fine because latent = max over s (permutation invariant).
  - rotate-out: p1 = g (.) cos2P, p2 = g (.) sin2P (plane-ordered tables)
  - y = CM1@p1 + CM2@p2 + D@obsT(strided) per 512-block; two blocks share
    one PSUM bank (partitions 0:64 / 64:128) so each max-reduce covers two
    blocks; final cross-partition pair-max via P matmul + tensor MAX.
  - tanh MLP head on [latent, act].
"""

import sys
import numpy as np
from contextlib import ExitStack

for _p in ("/opt/trn_rl_repo",):
    if _p not in sys.path:
        sys.path.insert(0, _p)

import ml_dtypes
import concourse.bass as bass
import concourse.tile as tile
from concourse import bacc, mybir
from concourse.bass_utils import run_bass_kernel_spmd

BF16 = mybir.dt.bfloat16
F32 = mybir.dt.float32

B_, S_, A_, D_IN, H_, D_OUT, D_MLP = 64, 2048, 128, 384, 64, 64, 64
NCORES = 8
NB = B_ // NCORES          # 8 batches per core
NSB = S_ // 512            # 4 s-blocks of 512
NDC = D_IN // 128          # 3 d-chunks
SH = S_ // 2               # 1024, scan length / plane width


def _build_nc():
    nc = bacc.Bacc("TRN2", target_bir_lowering=False, debug=False,
                   num_devices=1)

    # ---- DRAM I/O ----
    obsT_d = nc.dram_tensor("obsT", [NB, NDC, 128, S_], BF16,
                            kind="ExternalInput").ap()
    actT_d = nc.dram_tensor("actT", [NDC, 128, NB * A_], BF16,
                            kind="ExternalInput").ap()
    tabs_d = nc.dram_tensor("tabs", [128, 3 * S_], BF16, kind="ExternalInput").ap()
    rhopk_d = nc.dram_tensor("rhopk", [128, SH + 1], F32, kind="ExternalInput").ap()
    statpk_d = nc.dram_tensor("statpk", [128, 1600], BF16, kind="ExternalInput").ap()
    w2_d = nc.dram_tensor("w2", [64, 32], BF16, kind="ExternalInput").ap()
    w3_d = nc.dram_tensor("w3", [32, 1], BF16, kind="ExternalInput").ap()
    b1_d = nc.dram_tensor("b1", [64, 1], F32, kind="ExternalInput").ap()
    b2_d = nc.dram_tensor("b2", [32, 1], F32, kind="ExternalInput").ap()
    b3_d = nc.dram_tensor("b3", [1, 1], F32, kind="ExternalInput").ap()
    out_d = nc.dram_tensor("out", [1, NB * A_], F32, kind="ExternalOutput").ap()

    MULT = mybir.AluOpType.mult
    ADD = mybir.AluOpType.add
    MAX = mybir.AluOpType.max
    TANH = mybir.ActivationFunctionType.Tanh
    X = mybir.AxisListType.X

    with tile.TileContext(nc) as tc, ExitStack() as ctx:
        const = ctx.enter_context(tc.tile_pool(name="const", bufs=1))
        obsT_pool = ctx.enter_context(tc.tile_pool(name="obsT", bufs=4))
        work = ctx.enter_context(tc.tile_pool(name="work", bufs=2))
        tpool = ctx.enter_context(tc.tile_pool(name="tpool", bufs=3))
        pUA = ctx.enter_context(tc.tile_pool(name="pUA", bufs=1, space="PSUM"))
        pWE = ctx.enter_context(tc.tile_pool(name="pWE", bufs=1, space="PSUM"))
        pY = ctx.enter_context(tc.tile_pool(name="pY", bufs=1, space="PSUM"))
        small = ctx.enter_context(tc.tile_pool(name="small", bufs=1))

        def load_const(ap_d, shape, dtype, suffix=""):
            nm = f"c_{ap_d.tensor.name}{suffix}"
            t = const.tile(shape, dtype, tag=nm, name=nm)
            nc.scalar.dma_start(out=t[:], in_=ap_d)
            return t

        # packed consts: stationaries first (small, unblock compute), then
        # big tables split across both hwdge queues
        statpk = const.tile([128, 1600], BF16, tag="statpk", name="statpk")
        nc.scalar.dma_start(out=statpk[:], in_=statpk_d)
        tabs = const.tile([128, 3 * S_], BF16, tag="tabs", name="tabs")
        nc.scalar.dma_start(out=tabs[:, 0:S_], in_=tabs_d[:, 0:S_])
        nc.scalar.dma_start(out=tabs[:, S_:2 * S_], in_=tabs_d[:, S_:2 * S_])
        rhopk = const.tile([128, SH + 1], F32, tag="rhopk", name="rhopk")
        w2 = load_const(w2_d, [64, 32], BF16)
        w3 = load_const(w3_d, [32, 1], BF16)
        b1 = load_const(b1_d, [64, 1], F32)
        b2 = load_const(b2_d, [32, 1], F32)
        b3 = load_const(b3_d, [1, 1], F32)

        cosS = tabs[:, 0:S_]
        sinpm2 = tabs[:, S_:2 * S_]
        cosE = tabs[:, 2 * S_:2 * S_ + SH]
        sinE = tabs[:, 2 * S_ + SH:3 * S_]
        rho2f = rhopk[:, 0:SH]
        rho1 = rhopk[:, SH:SH + 1]
        statA = [statpk[:, k * 128:(k + 1) * 128] for k in range(NDC)]
        permP = statpk[:, 384:512]
        ident = statpk[:, 512:640]
        statD = [statpk[:, 640 + k * 64:640 + (k + 1) * 64] for k in range(NDC)]
        cm1 = statpk[:, 832:896]
        cm2 = statpk[:, 896:960]
        w1lat = statpk[:, 960:1024]
        w1act = [statpk[:, 1024 + k * 64:1024 + (k + 1) * 64] for k in range(NDC)]
        rhoI = statpk[:, 1216:1344]
        rhoP = statpk[:, 1344:1472]
        cm1l = statpk[:, 1472:1536]
        cm2l = statpk[:, 1536:1600]

        lat128 = small.tile([128, NB], F32)     # per-pair latent maxima

        # prefetch batch-0 obsT ahead of everything else on the sync queue
        obsT0 = [obsT_pool.tile([128, S_], BF16, tag=f"obsT{k}",
                                name=f"obsT{k}") for k in range(NDC)]
        for k in range(NDC):
            nc.sync.dma_start(out=obsT0[k][:], in_=obsT_d[0, k])

        # action-side MLP input (independent of the LRU path): compute
        # xa = W1act @ actT early so the tail only needs activations
        actT = [small.tile([128, NB * A_], BF16, tag=f"actT{k}",
                           name=f"actT{k}") for k in range(NDC)]
        for k in range(NDC):
            nc.scalar.dma_start(out=actT[k][:], in_=actT_d[k])
        nc.scalar.dma_start(out=rhopk[:], in_=rhopk_d)
        nc.sync.dma_start(out=tabs[:, 2 * S_:], in_=tabs_d[:, 2 * S_:])
        xa = small.tile([64, NB * A_], F32, tag="xa", name="xa")
        lat128b = small.tile([128, NB], BF16)
        latf = small.tile([64, NB], F32)
        latb = small.tile([64, NB], BF16)
        latWb = small.tile([64, NB], F32)
        x1 = small.tile([64, NB * A_], BF16)
        x2 = small.tile([32, NB * A_], BF16)
        x3 = small.tile([1, NB * A_], F32)

        def emit_mlp_half(h):
            bs = slice(h * (NB // 2), (h + 1) * (NB // 2))
            hl = slice(h * 512, (h + 1) * 512)
            nc.vector.tensor_copy(out=lat128b[:, bs], in_=lat128[:, bs])
            pswap = pWE.tile([128, 512], F32, tag="wE0", name="pswap")
            nc.tensor.matmul(out=pswap[:, 0:NB // 2], lhsT=permP,
                             rhs=lat128b[:, bs], start=True, stop=True)
            nc.vector.tensor_tensor(out=latf[:, bs], in0=lat128[0:64, bs],
                                    in1=pswap[0:64, 0:NB // 2], op=MAX)
            nc.vector.tensor_copy(out=latb[:, bs], in_=latf[:, bs])
            platW = pWE.tile([128, 512], F32, tag="wE1", name="platW")
            nc.tensor.matmul(out=platW[:64, 0:NB // 2], lhsT=w1lat[0:64, :],
                             rhs=latb[:, bs], start=True, stop=True)
            nc.vector.tensor_scalar(out=latWb[:, bs],
                                    in0=platW[:64, 0:NB // 2],
                                    scalar1=b1[:], scalar2=None, op0=ADD)
            for bb in range(NB // 2):
                b_idx = h * 4 + bb
                nc.scalar.activation(
                    out=x1[:, b_idx * A_:(b_idx + 1) * A_],
                    in_=xa[:, b_idx * A_:(b_idx + 1) * A_],
                    func=TANH, bias=latWb[:, b_idx:b_idx + 1], scale=1.0)
            px = pWE.tile([128, 512], F32, tag="wE0", name="px2")
            nc.tensor.matmul(out=px[:32, :], lhsT=w2[:], rhs=x1[:, hl],
                             start=True, stop=True)
            nc.scalar.activation(out=x2[:, hl], in_=px[:32, :], func=TANH,
                                 bias=b2[:], scale=1.0)
            px3 = pWE.tile([128, 512], F32, tag="wE1", name="px3")
            nc.tensor.matmul(out=px3[:1, :], lhsT=w3[:], rhs=x2[:, hl],
                             start=True, stop=True)
            nc.scalar.activation(out=x3[:, hl], in_=px3[:1, :], func=TANH,
                                 bias=b3[:], scale=1.0)

        def emit_xa():
            for half in range(2):
                hl = slice(half * 512, (half + 1) * 512)
                pxa = pWE.tile([128, 512], F32, tag="wE0", name="pxa")
                for k in range(NDC):
                    nc.tensor.matmul(out=pxa[:64, :], lhsT=w1act[k],
                                     rhs=actT[k][:, hl],
                                     start=(k == 0), stop=(k == NDC - 1))
                nc.scalar.copy(out=xa[:, hl], in_=pxa[:64, :])

        # ------- software-pipelined main loop: statA(b) | wE(b-1) | y(b-2)
        st = {}

        def stage_load(b):
            if b == 0:
                st[b] = {"obsT": obsT0}
                return
            obsT = [obsT_pool.tile([128, S_], BF16, tag=f"obsT{k}",
                                   name=f"obsT{k}") for k in range(NDC)]
            for k in range(NDC):
                nc.sync.dma_start(out=obsT[k][:], in_=obsT_d[b, k])
            st[b] = {"obsT": obsT}

        def stage_statA(b):
            s = st[b]
            uA = [None] * NSB
            for k in range(NDC):
                for i in range(NSB):
                    if k == 0:
                        uA[i] = pUA.tile([128, 512], F32, tag=f"uA{i}",
                                         name=f"uA{i}")
                    nc.tensor.matmul(
                        out=uA[i][:], lhsT=statA[k],
                        rhs=s["obsT"][k][:, i * 512:(i + 1) * 512],
                        start=(k == 0), stop=(k == NDC - 1))
            s["uA"] = uA

        def stage_rot(b):
            s = st[b]
            t1 = work.tile([128, S_ + 1], BF16, tag="t1", name="t1", bufs=3)
            t2 = work.tile([128, S_ + 1], BF16, tag="t2", name="t2", bufs=3)
            nc.gpsimd.memset(t1[:, 0:1], 0.0)
            nc.gpsimd.memset(t2[:, 0:1], 0.0)
            uAc = work.tile([128, S_], BF16, tag="uAc", name="uAc", bufs=3)
            for i in range(NSB):
                sl = slice(i * 512, (i + 1) * 512)
                slp = slice(1 + i * 512, 1 + (i + 1) * 512)
                nc.scalar.copy(out=uAc[:, sl], in_=s["uA"][i][:])
                nc.vector.tensor_tensor(out=t1[:, slp], in0=s["uA"][i][:],
                                        in1=cosS[:, sl], op=MULT)
                nc.gpsimd.tensor_tensor(out=t2[:, slp], in0=uAc[:, sl],
                                        in1=sinpm2[:, sl], op=MULT)
            s["t1"], s["t2"], s["uAc"] = t1, t2, uAc

        def stage_wE(b):
            s = st[b]
            t1, t2 = s["t1"], s["t2"]
            t1_lo = t1[:, 0:S_].rearrange("p (n f) -> p f n", f=2)[:, 0]
            t1_hi = t1[:, 1:S_ + 1].rearrange("p (n f) -> p f n", f=2)[:, 0]
            t2_lo = t2[:, 0:S_].rearrange("p (n f) -> p f n", f=2)[:, 0]
            t2_hi = t2[:, 1:S_ + 1].rearrange("p (n f) -> p f n", f=2)[:, 0]
            wE = [pWE.tile([128, 512], F32, tag=f"wE{j}", name=f"wE{j}")
                  for j in range(2)]
            for j in range(2):
                jl = slice(j * 512, (j + 1) * 512)
                nc.tensor.matmul(out=wE[j][:], lhsT=rhoI, rhs=t1_lo[:, jl],
                                 start=True, stop=False)
                nc.tensor.matmul(out=wE[j][:], lhsT=rhoP, rhs=t2_lo[:, jl],
                                 start=False, stop=False)
                nc.tensor.matmul(out=wE[j][:], lhsT=ident, rhs=t1_hi[:, jl],
                                 start=False, stop=False)
                nc.tensor.matmul(out=wE[j][:], lhsT=permP, rhs=t2_hi[:, jl],
                                 start=False, stop=True)
            s["wE"] = wE

        def stage_scan_p(b):
            s = st[b]
            g = work.tile([128, SH], BF16, tag="g", name="g")
            nc.vector.tensor_tensor_scan(out=g[:, 0:512],
                                         data0=rho2f[:, 0:512],
                                         data1=s["wE"][0][:], initial=0.0,
                                         op0=MULT, op1=ADD)
            nc.vector.tensor_tensor_scan(out=g[:, 512:SH],
                                         data0=rho2f[:, 512:SH],
                                         data1=s["wE"][1][:],
                                         initial=g[:, 511:512],
                                         op0=MULT, op1=ADD)
            p1 = work.tile([128, SH], BF16, tag="p1", name="p1", bufs=3)
            p2 = work.tile([128, SH], BF16, tag="p2", name="p2", bufs=3)
            nc.vector.tensor_tensor(out=p1[:], in0=g[:], in1=cosE[:], op=MULT)
            nc.vector.tensor_tensor(out=p2[:], in0=g[:], in1=sinE[:], op=MULT)
            s["p1"], s["p2"] = p1, p2

        def stage_y(b):
            s = st[b]
            py = [pY.tile([128, 512], F32, tag=f"pY{j}", name=f"pY{j}")
                  for j in range(2)]
            subs = [(pl, blk) for pl in range(2) for blk in range(2)]

            def sub_out(pl, blk):
                return py[blk][pl * 64:(pl + 1) * 64, :]

            uAc_odd = s["uAc"][:].rearrange("p (n f) -> p f n", f=2)[:, 1]
            for pl, blk in subs:
                jl = slice(blk * 512, (blk + 1) * 512)
                nc.tensor.matmul(out=sub_out(pl, blk),
                                 lhsT=(cm1 if pl == 0 else cm1l),
                                 rhs=s["p1"][:, jl], start=True, stop=False)
                nc.tensor.matmul(out=sub_out(pl, blk),
                                 lhsT=(cm2 if pl == 0 else cm2l),
                                 rhs=s["p2"][:, jl], start=False, stop=False)
                if pl == 1:
                    nc.tensor.matmul(out=sub_out(pl, blk), lhsT=cm1,
                                     rhs=uAc_odd[:, jl], start=False,
                                     stop=False)
                for k in range(NDC):
                    base = s["obsT"][k][:, blk * 1024:(blk + 1) * 1024]
                    obsP = base.rearrange("p (n f) -> p f n", f=2)[:, pl]
                    nc.tensor.matmul(out=sub_out(pl, blk), lhsT=statD[k],
                                     rhs=obsP, start=False,
                                     stop=(k == NDC - 1))
            s["py"] = py

        def stage_red(b):
            s = st[b]
            ymax = small.tile([128, 2], F32, tag="ymax", name="ymax")
            for j in range(2):
                nc.vector.tensor_reduce(out=ymax[:, j:j + 1],
                                        in_=s["py"][j][:], axis=X, op=MAX)
            nc.vector.tensor_reduce(out=lat128[:, b:b + 1], in_=ymax[:],
                                    axis=X, op=MAX)
            del st[b]

        for i in range(NB + 2):
            if i < NB:
                stage_load(i)
                stage_statA(i)
                stage_rot(i)
            if 1 <= i <= NB:
                stage_wE(i - 1)
                stage_scan_p(i - 1)
            if i >= 2:
                stage_y(i - 2)
                stage_red(i - 2)
            if i == 0:
                emit_xa()
            if i == NB // 2 + 1:
                emit_mlp_half(0)
        emit_mlp_half(1)

        nc.sync.dma_start(out=out_d, in_=x3[:])

    nc.compile()
    return nc


_NC_CACHE = {}


def _get_nc():
    if "nc" not in _NC_CACHE:
        _NC_CACHE["nc"] = _build_nc()
    return _NC_CACHE["nc"]


def _host_tables(nu_log, theta_log, gamma_log, B_re, B_im, C_re, C_im, D,
                 W1, b1, W2, b2, W3, b3):
    f64 = np.float64
    bf = ml_dtypes.bfloat16
    rho_h = np.exp(-np.exp(nu_log.astype(f64)))          # [H]
    theta_h = np.exp(theta_log.astype(f64))              # [H]
    gamma_h = np.exp(gamma_log.astype(f64))              # [H]
    s = np.arange(S_, dtype=f64)
    phase = (theta_h[:, None] * s[None, :]) % (2 * np.pi)   # [H, S]
    cos_t = np.cos(phase)
    sin_t = np.sin(phase)

    def dup(x):  # [H,S] -> [128,S]
        return np.concatenate([x, x], axis=0)

    cosS = dup(cos_t).astype(bf)
    sinpm2 = np.concatenate([-sin_t, sin_t], axis=0).astype(bf)
    cosE = dup(cos_t)[:, 0::2].astype(bf)
    sinE = dup(sin_t)[:, 0::2].astype(bf)

    rho128 = np.concatenate([rho_h, rho_h]).astype(f64)     # [128]
    rho1 = rho128.astype(np.float32).reshape(128, 1)
    rho2f = np.broadcast_to((rho128 ** 2).astype(np.float32)[:, None],
                            (128, SH)).copy()

    Bg_re = (B_re.astype(f64) * gamma_h[:, None])        # [H, D_IN]
    Bg_im = (B_im.astype(f64) * gamma_h[:, None])
    statA = np.concatenate([Bg_re.T, Bg_im.T], axis=1)   # [D_IN, 128]
    statA = statA.reshape(NDC, 128, 128).astype(bf)
    permP = np.zeros((128, 128), dtype=bf)
    for m in range(128):
        permP[m ^ 64, m] = 1
    ident = np.eye(128, dtype=bf)
    statD = D.T.reshape(NDC, 128, D_OUT).astype(bf)

    cm1 = np.concatenate([C_re.T, -C_im.T], axis=0).astype(bf)
    cm2 = np.concatenate([-C_im.T, -C_re.T], axis=0).astype(bf)
    inC_re = C_re.astype(f64)
    inC_im = C_im.astype(f64)

    w1lat = np.zeros((128, 64), dtype=np.float64)
    w1lat[:H_] = W1[:, :H_].T
    w1lat = w1lat.astype(bf)                             # [128, 64] padded
    w1act = W1[:, H_:].T.reshape(NDC, 128, D_MLP).astype(bf)
    w2 = W2.T.astype(bf)                                 # [64, 32]
    w3 = W3.T.astype(bf)                                 # [32, 1]

    tabs = np.concatenate([cosS, sinpm2, cosE, sinE], axis=1)
    rhopk = np.concatenate([rho2f, rho1], axis=1).astype(np.float32)
    # statpk layout: statA(3x128) permP ident statD(3x64) cm1 cm2 w1lat w1act(3x64)
    # lambda-folded C: C' = C * diag(lambda)
    lam_re = (rho128[:H_] * np.cos(theta_h))
    lam_im = (rho128[:H_] * np.sin(theta_h))
    Cp_re = inC_re * lam_re[None, :] - inC_im * lam_im[None, :]
    Cp_im = inC_re * lam_im[None, :] + inC_im * lam_re[None, :]
    cm1l = np.concatenate([Cp_re.T, -Cp_im.T], axis=0).astype(bf)
    cm2l = np.concatenate([-Cp_im.T, -Cp_re.T], axis=0).astype(bf)
    rhoI = (np.eye(128) * rho128[None, :]).astype(bf)
    rhoP = (permP.astype(np.float64) * rho128[None, :]).astype(bf)
    statpk = np.concatenate(
        [np.concatenate([statA[k] for k in range(NDC)], axis=1),
         permP, ident,
         np.concatenate([statD[k] for k in range(NDC)], axis=1),
         cm1, cm2, w1lat,
         np.concatenate([w1act[k] for k in range(NDC)], axis=1),
         rhoI, rhoP, cm1l, cm2l],
        axis=1).astype(bf)
    assert statpk.shape == (128, 1600), statpk.shape
    return dict(
        tabs=tabs, rhopk=rhopk, statpk=statpk,
        w2=w2, w3=w3,
        b1=b1.reshape(64, 1).astype(np.float32),
        b2=b2.reshape(32, 1).astype(np.float32),
        b3=b3.reshape(1, 1).astype(np.float32),
    )


def kernel(observations, actions, nu_log, theta_log, gamma_log,
           B_re, B_im, C_re, C_im, D, W1, b1, W2, b2, W3, b3,
           _trace=False, _tmpdir=None):
    obs_bf = np.asarray(observations, dtype=np.float32).astype(
        ml_dtypes.bfloat16)
    act_bf = np.asarray(actions, dtype=np.float32).astype(ml_dtypes.bfloat16)
    # host-side transposes: obsT [B, NDC, 128, S]
    obsT_all = np.ascontiguousarray(obs_bf.transpose(0, 2, 1)).reshape(
        B_, NDC, 128, S_)
    tables = _host_tables(np.asarray(nu_log), np.asarray(theta_log),
                          np.asarray(gamma_log), np.asarray(B_re),
                          np.asarray(B_im), np.asarray(C_re),
                          np.asarray(C_im), np.asarray(D),
                          np.asarray(W1), np.asarray(b1), np.asarray(W2),
                          np.asarray(b2), np.asarray(W3), np.asarray(b3))
    in_maps = []
    for c in range(NCORES):
        m = dict(tables)
        m["obsT"] = np.ascontiguousarray(obsT_all[c * NB:(c + 1) * NB])
        act_c = act_bf[c * NB:(c + 1) * NB].reshape(NB * A_, D_IN)
        m["actT"] = np.ascontiguousarray(act_c.T).reshape(NDC, 128, NB * A_)
        in_maps.append(m)

    nc = _get_nc()
    res = run_bass_kernel_spmd(nc, in_maps, core_ids=list(range(NCORES)),
                               trace=_trace, tmpdir=_tmpdir)
    outs = []
    for c in range(NCORES):
        outs.append(np.asarray(res.results[c]["out"]).reshape(NB, A_, 1))
    full = np.concatenate(outs, axis=0).astype(np.float32)
    if _trace:
        return full, res
    return full
